# revision 24
# baseline (speedup 1.0000x reference)
"""Trainium2 Bass kernel: causal multi-head attention block (B=2,S=2048,H=2048,NH=16,HD=128).

Sharding: 8 cores = DP over batch (2) x TP over head-groups (4 groups of 4 heads).
Each core computes q/k/v projections for its 4 heads, RoPE, causal softmax
attention, and a partial output projection; the host sums the 4 partials per
batch and adds bo.

v2: single fused pipeline. For causal attention, q-tile i only attends k-tiles
<= i, so one pass over the 4 s-tiles of 512 suffices: iteration i runs
attention+output-projection for q-tile i while the projection chains for
s-tile i+1 are interleaved as PE filler work. This keeps the tensor engine
dense (no >1us gaps -> HAM stays at 2.4GHz; the 3-phase baseline spent 208us
at half clock), loads x once instead of twice, and turns every DMA into a
contiguous pre-arranged block. Softmax denominators come from an f16 running
sum of the exp tiles on the DVE plus a single ones-matmul per (head, q-tile)
(instead of a ones-matmul per k-tile: -62k PE cycles). y is stored f16.

Device layouts (host pre-arranges; matmul operands bf16, accumulation f32):
  x_pre  [4, 128, 16, 512]  x[b].T tiled [sj][fpart][ftile][s]      bf16
  wq/wk/wv [128, 16, 512]   W.T tiled [fpart][ftile][d]             bf16
  wo     [128, 4, 2048]     Wo[:,rows].T tiled [dpart][dtile][o]    bf16
  cos/sin [4, 128, 512]     RoPE tables per s-tile [d][s]           f32
  y      [4, 4, 128, 2048]  [qj][ss][qpart][o]                      f16
Attention uses transposed score tiles ST[k,q] so P@V needs no transposes;
1/denom is broadcast across partitions with a K=1 matmul.
"""

import math
import os
import sys

import numpy as np

for _p in ("/opt/trn_rl_repo",):
    if _p not in sys.path and os.path.isdir(_p):
        sys.path.insert(0, _p)

import ml_dtypes

import concourse.bass as bass
import concourse.mybir as mybir
import concourse.tile as tile
from concourse import bacc

B, S, H, NH, HD = 2, 2048, 2048, 16, 128
NCORES = 8
HG = 4            # head-groups (TP degree)
HPG = NH // HG    # heads per group = 4
DLOC = HPG * HD   # local d width = 512
FT = H // 128     # 16 f-tiles
SJ = S // 512     # 4 s/q tiles of 512
KT128 = S // 128  # 16 k-tiles of 128
NEG = -1e30

F32 = mybir.dt.float32
F16 = mybir.dt.float16
BF16 = mybir.dt.bfloat16
NPBF16 = ml_dtypes.bfloat16
AF = mybir.ActivationFunctionType


def build_program(mode: str) -> bass.Bass:
    """mode in {'causal', 'full', 'bias'}"""
    # Tile enforces FIFO retirement per DMA-completion sem lane by making each
    # DMA issue wait for the previous one on its lane: with 1 lane every DMA
    # serializes on the ~2.2us completion round-trip (measured: the whole
    # bootstrap burst trickled at one transfer per 2.2us). 8 lanes allow 8
    # DMAs in flight. (The old "Too many sync wait commands" overflow that
    # forced 1 lane in the phase-structured kernel does not trigger with this
    # program shape.)
    import concourse.tile_sem_assignment as tsa

    tsa.NUM_HWDGE_SEMS = 8
    tsa.NUM_SWDGE_GLOBAL_SEMS = 1
    nc = bacc.Bacc()
    x_pre = nc.dram_tensor("x_pre", [SJ, 128, FT, 512], BF16, kind="ExternalInput")
    wq = nc.dram_tensor("wq", [HPG, 128, FT, 128], BF16, kind="ExternalInput")
    wk = nc.dram_tensor("wk", [HPG, 128, FT, 128], BF16, kind="ExternalInput")
    wv = nc.dram_tensor("wv", [128, FT, DLOC], BF16, kind="ExternalInput")
    wo = nc.dram_tensor("wo", [128, HPG, H], BF16, kind="ExternalInput")
    bqT = nc.dram_tensor("bqT", [128, HPG], F32, kind="ExternalInput")
    bkT = nc.dram_tensor("bkT", [128, HPG], F32, kind="ExternalInput")
    bvb = nc.dram_tensor("bvb", [128, DLOC], BF16, kind="ExternalInput")
    cosp = nc.dram_tensor("cosp", [SJ, HD, 512], F32, kind="ExternalInput")
    sinp = nc.dram_tensor("sinp", [SJ, HD, 512], F32, kind="ExternalInput")
    rmat = nc.dram_tensor("rmat", [HD, HD], BF16, kind="ExternalInput")
    onesd = nc.dram_tensor("onesd", [128, 1], F16, kind="ExternalInput")
    onesrd = nc.dram_tensor("onesrd", [1, 128], F16, kind="ExternalInput")
    if mode == "causal":
        dbias = nc.dram_tensor("dbias", [128, 4, 512], BF16, kind="ExternalInput")
    elif mode == "bias":
        fbias = nc.dram_tensor("fbias", [S, S], F32, kind="ExternalInput")
    y = nc.dram_tensor("y", [SJ, 4, 128, H], F16, kind="ExternalOutput")

    causal = mode == "causal"

    from contextlib import ExitStack

    with tile.TileContext(nc) as tc:
        with ExitStack() as ctx:
            pool = lambda *a, **k: ctx.enter_context(tc.tile_pool(*a, **k))  # noqa: E731
            consts = pool(name="consts", bufs=1)
            wqp = pool(name="wqp", bufs=1)
            wkp = pool(name="wkp", bufs=1)
            wvp = pool(name="wvp", bufs=1)
            wop = pool(name="wop", bufs=1)
            xin = pool(name="xin", bufs=2)
            cs_pool = pool(name="cs", bufs=2)
            qt_pool = pool(name="qt", bufs=9 if causal else KT128)
            kt_pool = pool(name="kt", bufs=KT128)
            vt_pool = pool(name="vt", bufs=KT128)
            rtmp_pool = pool(name="rtmp", bufs=2)
            e_pool = pool(name="ep", bufs=5)
            ee_pool = pool(name="eep", bufs=2)
            rc_pool = pool(name="rc", bufs=2)
            ot_pool = pool(name="ot", bufs=9)
            y_pool = pool(name="ysb", bufs=3)
            fb_pool = pool(name="fb", bufs=3)
            acc_psum = pool(name="acc", bufs=2, space="PSUM")
            st_psum = pool(name="stp", bufs=2, space="PSUM")
            pv_psum = pool(name="pvp", bufs=2, space="PSUM")
            yp_psum = pool(name="ypp", bufs=2, space="PSUM")
            rmat_sb = consts.tile([HD, HD], BF16, tag="rmat")
            bq_sb = consts.tile([128, HPG], F32, tag="bq")
            bk_sb = consts.tile([128, HPG], F32, tag="bk")
            ones_sb = consts.tile([128, 1], F16, tag="ones")
            onesr_sb = consts.tile([1, 128], F16, tag="onesr")

            # ---- bootstrap DMAs, ordered to match PE consumption order:
            # q-chains by head (wq per-head chunks interleaved with x0
            # quarters), then RoPE tables, k h0, the full wv (attention(0)
            # needs all V tiles first), remaining wk heads, then the
            # iteration-0 prefetches. ----
            wq_sb = wqp.tile([128, HPG, FT, 128], BF16, tag="wq")
            wk_sb = wkp.tile([128, HPG, FT, 128], BF16, tag="wk")
            wv_sb = wvp.tile([128, FT, DLOC], BF16, tag="wv")
            wo_sb = wop.tile([128, HPG, H], BF16, tag="wo")
            bv_sb = consts.tile([128, DLOC], BF16, tag="bv")
            db_sb = None
            if causal:
                db_sb = consts.tile([128, 4, 512], BF16, tag="db")

            XT = {}
            COS = {}
            SIN = {}
            XT[0] = xin.tile([128, FT, 512], BF16, tag="xt", name="xt")
            nc.sync.dma_start(wq_sb[:, 0], wq[0])
            nc.sync.dma_start(XT[0][:, 0:4, :], x_pre[0, :, 0:4, :])
            nc.sync.dma_start(XT[0][:, 4:8, :], x_pre[0, :, 4:8, :])
            # tiny consts ride behind the first chain's inputs
            nc.sync.dma_start(rmat_sb[:], rmat[:])
            nc.sync.dma_start(bq_sb[:], bqT[:])
            nc.sync.dma_start(bk_sb[:], bkT[:])
            nc.sync.dma_start(ones_sb[:], onesd[:])
            nc.sync.dma_start(onesr_sb[:], onesrd[:])
            nc.sync.dma_start(wq_sb[:, 1], wq[1])
            nc.sync.dma_start(XT[0][:, 8:12, :], x_pre[0, :, 8:12, :])
            nc.sync.dma_start(wq_sb[:, 2], wq[2])
            nc.sync.dma_start(XT[0][:, 12:16, :], x_pre[0, :, 12:16, :])
            nc.sync.dma_start(wq_sb[:, 3], wq[3])
            COS[0] = cs_pool.tile([HD, 512], F32, tag="cos", name="cos")
            SIN[0] = cs_pool.tile([HD, 512], F32, tag="sin", name="sin")
            nc.sync.dma_start(COS[0][:], cosp[0])
            nc.sync.dma_start(SIN[0][:], sinp[0])
            nc.sync.dma_start(wk_sb[:, 0], wk[0])
            nc.sync.dma_start(bv_sb[:], bvb[:])
            nc.sync.dma_start(wv_sb[:], wv[:])
            for hh in range(1, HPG):
                nc.sync.dma_start(wk_sb[:, hh], wk[hh])
            if causal:
                nc.sync.dma_start(db_sb[:], dbias[:])
            # prefetch s-tile 1 inputs during the bootstrap projections
            XT[1] = xin.tile([128, FT, 512], BF16, tag="xt", name="xt")
            nc.sync.dma_start(XT[1][:], x_pre[1])
            COS[1] = cs_pool.tile([HD, 512], F32, tag="cos", name="cos")
            SIN[1] = cs_pool.tile([HD, 512], F32, tag="sin", name="sin")
            nc.sync.dma_start(COS[1][:], cosp[1])
            nc.sync.dma_start(SIN[1][:], sinp[1])
            nc.sync.dma_start(wo_sb[:], wo[:])

            QT = {}   # (h, sj) -> [128(d), 512(s)] bf16, RoPE'd q^T (pre-scaled)
            KT = {}   # (h, sj) -> [128(d), 512(s)] bf16, RoPE'd k^T
            VT = {}   # ss -> [128(s), 512(d)] bf16, v + bias
            PV = {}   # h -> psum [128(d), 512(q)] accumulated P@V
            EE = {}   # h -> [128(k), 512(q)] f16 running sum of exp tiles
            RCH = {}  # h -> [1, 512] f16 reciprocal denominators
            OT = {}   # h -> [128(d), 512(q)] bf16 normalized attention out

            def proj_qk(which, h, i):
                """Projection chain + RoPE for (q|k, head h, s-tile i)."""
                w_sb, b_sb = (wq_sb, bq_sb) if which == "q" else (wk_sb, bk_sb)
                store = QT if which == "q" else KT
                pool = qt_pool if which == "q" else kt_pool
                ps = acc_psum.tile([128, 512], F32, tag="acc", name="acc")
                for ft in range(FT):
                    nc.tensor.matmul(
                        ps[:],
                        w_sb[:, h, ft, :],
                        XT[i][:, ft, :],
                        start=(ft == 0),
                        stop=(ft == FT - 1),
                    )
                t = pool.tile([128, 512], BF16, tag="t", name="qkt")
                nc.scalar.activation(t[:], ps[:], AF.Identity, bias=b_sb[:, h : h + 1])
                rp = st_psum.tile([128, 512], F32, tag="st", name="rp")
                nc.tensor.matmul(rp[:], rmat_sb[:], t[:], start=True, stop=True)
                tmp = rtmp_pool.tile([128, 512], BF16, tag="tmp", name="tmp")
                nc.vector.tensor_mul(tmp[:], rp[:], SIN[i][:])
                nc.vector.tensor_mul(t[:], t[:], COS[i][:])
                nc.vector.tensor_add(t[:], t[:], tmp[:])
                store[(h, i)] = t

            def proj_v(u, i):
                """V projection for s-subtile 128*(4i+u)."""
                ss = 4 * i + u
                ps = acc_psum.tile([128, 512], F32, tag="acc", name="acc")
                for ft in range(FT):
                    nc.tensor.matmul(
                        ps[:],
                        XT[i][:, ft, u * 128 : (u + 1) * 128],
                        wv_sb[:, ft, :],
                        start=(ft == 0),
                        stop=(ft == FT - 1),
                    )
                v = vt_pool.tile([128, DLOC], BF16, tag="v", name="v")
                nc.vector.tensor_add(v[:], ps[:], bv_sb[:])
                VT[ss] = v

            def emit_unit(unit, i):
                which, idx = unit
                if which == "v":
                    proj_v(idx, i)
                else:
                    proj_qk(which, idx, i)

            def denom(h):
                """Start the denominator chain for head h (dn matmul+recip).

                dn lives in the acc pool: during attention the projection
                accumulators are mostly idle, while the st pool's two banks
                are kept saturated by the ST pipeline."""
                dn = acc_psum.tile([1, 512], F32, tag="acc", name="dn")
                nc.tensor.matmul(dn[:], ones_sb[:], EE[h][:], start=True, stop=True)
                rcf = rc_pool.tile([1, 512], F32, tag="rcf", name="rcf")
                nc.vector.reciprocal_approx_fast(rcf[:], dn[:])
                rch = rc_pool.tile([1, 512], F16, tag="rch", name="rch")
                nc.vector.tensor_copy(rch[:], rcf[:])
                RCH[h] = rch

            def normalize(i, h):
                """Broadcast 1/denom and scale P@V for head h."""
                rcb_ps = yp_psum.tile([128, 512], F32, tag="yp", name="rcb_ps")
                nc.tensor.matmul(rcb_ps[:], onesr_sb[:], RCH[h][:], start=True, stop=True)
                rcb = rc_pool.tile([128, 512], F16, tag="rcb", name="rcb")
                nc.vector.tensor_copy(rcb[:], rcb_ps[:])
                ot = ot_pool.tile([128, 512], BF16, tag="ot", name="ot")
                nc.vector.tensor_mul(ot[:], PV[h][:], rcb[:])
                OT[(i, h)] = ot

            def attn_head(i, h, fillers=None):
                """Causal attention for (q-tile i, head h); ST[k,q] layout.

                The denom/normalize work for head h-1 is sprinkled after
                kj==0 and kj==1 so the PE never waits on the DVE reciprocal
                chain.  `fillers` is a list of callables emitting independent
                PE work (deferred oproj chains); one is drained every 4
                k-tiles to cover the ~200ns/k-tile exp-gated PV stall in the
                last iteration.
                """
                kmax = 4 * i + 4 if causal else KT128
                pv = pv_psum.tile([128, 512], F32, tag="pv", name="pv")
                ee = ee_pool.tile([128, 512], F16, tag="ee", name="ee")
                for kj in range(kmax):
                    a = kj - 4 * i
                    off = 128 * a if (causal and a > 0) else 0
                    st = st_psum.tile([128, 512], F32, tag="st", name="st")
                    nc.tensor.matmul(
                        st[:, off:],
                        KT[(h, kj // 4)][:, (kj % 4) * 128 : (kj % 4 + 1) * 128],
                        QT[(h, i)][:, off:],
                        start=True,
                        stop=True,
                    )
                    if causal and a >= 0:
                        nc.vector.tensor_add(
                            st[:, off : off + 128],
                            st[:, off : off + 128],
                            db_sb[:, a, off : off + 128],
                        )
                    elif mode == "bias":
                        fb = fb_pool.tile([128, 512], F32, tag="fb", name="fb")
                        nc.sync.dma_start(
                            fb[:],
                            fbias[kj * 128 : (kj + 1) * 128, i * 512 : (i + 1) * 512],
                        )
                        nc.vector.tensor_add(st[:], st[:], fb[:])
                    e = e_pool.tile([128, 512], BF16, tag="e", name="e")
                    nc.scalar.activation(e[:, off:], st[:, off:], AF.Exp)
                    nc.tensor.matmul(
                        pv[:, off:],
                        VT[kj][:, h * 128 : (h + 1) * 128],
                        e[:, off:],
                        start=(kj == 0),
                        stop=(kj == kmax - 1),
                    )
                    if kj == 0:
                        nc.vector.tensor_copy(ee[:], e[:])
                    else:
                        nc.vector.tensor_add(ee[:, off:], ee[:, off:], e[:, off:])
                    if h > 0 and kj == 0:
                        denom(h - 1)
                    if h > 0 and kj == 1:
                        normalize(i, h - 1)
                    if fillers and kj % 4 == 3:
                        fillers.pop(0)()
                PV[h] = pv
                EE[h] = ee

            def oproj(i, ss):
                """Output projection rows 512i+128ss..+128, all 2048 cols."""
                ysb = y_pool.tile([128, H], F16, tag="y", name="y")
                for oj in range(4):
                    yp = yp_psum.tile([128, 512], F32, tag="yp", name="yp")
                    for dt in range(HPG):
                        nc.tensor.matmul(
                            yp[:],
                            OT[(i, dt)][:, ss * 128 : (ss + 1) * 128],
                            wo_sb[:, dt, oj * 512 : (oj + 1) * 512],
                            start=(dt == 0),
                            stop=(dt == HPG - 1),
                        )
                    dst = ysb[:, oj * 512 : (oj + 1) * 512]
                    if oj % 2 == 0:
                        nc.scalar.activation(dst, yp[:], AF.Identity)
                    else:
                        nc.vector.tensor_copy(dst, yp[:])
                nc.sync.dma_start(y[i, ss], ysb[:])

            def oproj_pieces(i):
                """oproj(i) as 16 single (ss, oj) chain emissions for use as
                in-loop PE filler."""
                holder = {}

                def make(ss, oj):
                    def f():
                        if oj == 0:
                            holder[ss] = y_pool.tile([128, H], F16, tag="y", name="y")
                        ysb = holder[ss]
                        yp = yp_psum.tile([128, 512], F32, tag="yp", name="yp")
                        for dt in range(HPG):
                            nc.tensor.matmul(
                                yp[:],
                                OT[(i, dt)][:, ss * 128 : (ss + 1) * 128],
                                wo_sb[:, dt, oj * 512 : (oj + 1) * 512],
                                start=(dt == 0),
                                stop=(dt == HPG - 1),
                            )
                        dst = ysb[:, oj * 512 : (oj + 1) * 512]
                        if oj % 2 == 0:
                            nc.scalar.activation(dst, yp[:], AF.Identity)
                        else:
                            nc.vector.tensor_copy(dst, yp[:])
                        if oj == 3:
                            nc.sync.dma_start(y[i, ss], ysb[:])

                    return f

                return [make(ss, oj) for ss in range(4) for oj in range(4)]

            # ---- projections for s-tile 0, in DMA-arrival order ----
            for h in range(HPG):
                proj_qk("q", h, 0)
            proj_qk("k", 0, 0)
            for u in range(4):
                proj_v(u, 0)
            for h in range(1, HPG):
                proj_qk("k", h, 0)

            # For full/bias modes every q-tile needs all k-tiles, so emit all
            # projections up front and skip the interleaving.
            if not causal:
                for i in range(1, SJ):
                    XT[i] = XT.get(i) or xin.tile(
                        [128, FT, 512], BF16, tag="xt", name="xt"
                    )
                    if i > 1:
                        nc.sync.dma_start(XT[i][:], x_pre[i])
                        COS[i] = cs_pool.tile([HD, 512], F32, tag="cos", name="cos")
                        SIN[i] = cs_pool.tile([HD, 512], F32, tag="sin", name="sin")
                        nc.sync.dma_start(COS[i][:], cosp[i])
                        nc.sync.dma_start(SIN[i][:], sinp[i])
                    for h in range(HPG):
                        proj_qk("q", h, i)
                    for h in range(HPG):
                        proj_qk("k", h, i)
                    for u in range(4):
                        proj_v(u, i)

            UNITS = [("q", 0), ("q", 1), ("q", 2), ("q", 3),
                     ("k", 0), ("k", 1), ("k", 2), ("k", 3),
                     ("v", 0), ("v", 1), ("v", 2), ("v", 3)]

            for i in range(SJ):
                nxt = i + 1
                fill = causal and nxt < SJ
                if fill:
                    # prefetch s-tile i+2 inputs; the i+1 tiles landed during
                    # the previous iteration.
                    if nxt + 1 < SJ:
                        j = nxt + 1
                        COS[j] = cs_pool.tile([HD, 512], F32, tag="cos", name="cos")
                        SIN[j] = cs_pool.tile([HD, 512], F32, tag="sin", name="sin")
                        nc.sync.dma_start(COS[j][:], cosp[j])
                        nc.sync.dma_start(SIN[j][:], sinp[j])
                        XT[j] = xin.tile([128, FT, 512], BF16, tag="xt", name="xt")
                        nc.sync.dma_start(XT[j][:], x_pre[j])

                # oproj for q-tile i-1 is deferred into this iteration: its
                # matmul chains have no ACT/DVE dependencies, so they fill the
                # PE while the exp chain gates this tile's PV matmuls.  In the
                # last iteration (no projection units left) the pieces go
                # INSIDE the kj loops, where the exp-gated stalls actually are.
                last = causal and i == SJ - 1
                fillers = oproj_pieces(i - 1) if last else None
                attn_head(i, 0, fillers)
                if causal and 0 < i < SJ - 1:
                    oproj(i - 1, 0)
                if fill:
                    emit_unit(UNITS[0], nxt)
                attn_head(i, 1, fillers)
                if causal and 0 < i < SJ - 1:
                    oproj(i - 1, 1)
                if fill:
                    emit_unit(UNITS[1], nxt)
                attn_head(i, 2, fillers)
                if causal and 0 < i < SJ - 1:
                    oproj(i - 1, 2)
                if fill:
                    emit_unit(UNITS[2], nxt)
                    emit_unit(UNITS[3], nxt)
                attn_head(i, 3, fillers)
                if causal and 0 < i < SJ - 1:
                    oproj(i - 1, 3)
                if fill:
                    emit_unit(UNITS[4], nxt)
                denom(3)
                if fill:
                    emit_unit(UNITS[5], nxt)
                normalize(i, 3)
                if not causal:
                    for ss in range(4):
                        oproj(i, ss)
                if fill:
                    emit_unit(UNITS[6], nxt)
                    emit_unit(UNITS[7], nxt)
                    emit_unit(UNITS[8], nxt)
                    emit_unit(UNITS[9], nxt)
                    emit_unit(UNITS[10], nxt)
                    emit_unit(UNITS[11], nxt)
            if causal:
                for ss in range(4):
                    oproj(SJ - 1, ss)
    nc.compile()
    return nc


_PROGRAM_CACHE = {}


def _get_program(mode):
    if mode not in _PROGRAM_CACHE:
        _PROGRAM_CACHE[mode] = build_program(mode)
    return _PROGRAM_CACHE[mode]


def _detect_mode(attn_mask):
    m = np.asarray(attn_mask).reshape(S, S)
    if (m == np.tril(np.ones((S, S), m.dtype))).all():
        return "causal"
    if (m != 0).all():
        return "full"
    return "bias"


def _rot_matrix():
    # rot(q)[d'] = -q[d'+1] (d' even), +q[d'-1] (d' odd);  rotT = R^T @ qT with
    # lhsT[d, d'] convention of nc.tensor.matmul.
    r = np.zeros((HD, HD), np.float32)
    for dp in range(HD):
        if dp % 2 == 0:
            r[dp + 1, dp] = -1.0
        else:
            r[dp - 1, dp] = 1.0
    return r


def _diag_bias():
    # [p, a, t]: 0 where k=128a+p visible to q=t within the diagonal band
    a = np.arange(4)[None, :, None]
    p = np.arange(128)[:, None, None]
    t = np.arange(512)[None, None, :]
    return np.where(128 * a + p <= t, 0.0, NEG).astype(NPBF16)


def _bf16(a):
    return np.ascontiguousarray(a).astype(NPBF16)


def kernel(**inputs) -> np.ndarray:
    from concourse.bass_utils import run_bass_kernel_spmd

    x = np.asarray(inputs["x"], np.float32)
    fcos = np.asarray(inputs["fcos"], np.float32)
    fsin = np.asarray(inputs["fsin"], np.float32)
    Wq, bq = np.asarray(inputs["Wq"], np.float32), np.asarray(inputs["bq"], np.float32)
    Wk, bk = np.asarray(inputs["Wk"], np.float32), np.asarray(inputs["bk"], np.float32)
    Wv, bv = np.asarray(inputs["Wv"], np.float32), np.asarray(inputs["bv"], np.float32)
    Wo, bo = np.asarray(inputs["Wo"], np.float32), np.asarray(inputs["bo"], np.float32)
    attn_mask = inputs["attn_mask"]

    mode = _detect_mode(attn_mask)
    nc = _get_program(mode)

    sc = 1.0 / math.sqrt(HD)
    shared = {
        "cosp": np.ascontiguousarray(
            fcos.T.reshape(HD, SJ, 512).transpose(1, 0, 2)
        ).astype(np.float32),
        "sinp": np.ascontiguousarray(
            fsin.T.reshape(HD, SJ, 512).transpose(1, 0, 2)
        ).astype(np.float32),
        "rmat": _rot_matrix().astype(NPBF16),
        "onesd": np.ones((128, 1), np.float16),
        "onesrd": np.ones((1, 128), np.float16),
    }
    if mode == "causal":
        shared["dbias"] = _diag_bias()
    elif mode == "bias":
        m = np.asarray(attn_mask).reshape(S, S)
        shared["fbias"] = np.ascontiguousarray(
            np.where(m.T == 0, NEG, 0.0).astype(np.float32)
        )

    in_maps = []
    for c in range(NCORES):
        b, hg = divmod(c, HG)
        rows = slice(DLOC * hg, DLOC * (hg + 1))
        xT = x[b].T  # [H, S]
        in_maps.append(
            {
                "x_pre": _bf16(
                    xT.reshape(FT, 128, SJ, 512).transpose(2, 1, 0, 3)
                ),
                "wq": _bf16(
                    (Wq[rows] * sc).T.reshape(FT, 128, HPG, 128).transpose(2, 1, 0, 3)
                ),
                "wk": _bf16(
                    Wk[rows].T.reshape(FT, 128, HPG, 128).transpose(2, 1, 0, 3)
                ),
                "wv": _bf16(Wv[rows].T.reshape(FT, 128, DLOC).transpose(1, 0, 2)),
                "wo": _bf16(Wo[:, rows].T.reshape(HPG, 128, H).transpose(1, 0, 2)),
                "bqT": np.ascontiguousarray((bq[rows] * sc).reshape(HPG, 128).T),
                "bkT": np.ascontiguousarray(bk[rows].reshape(HPG, 128).T),
                "bvb": _bf16(
                    np.broadcast_to(bv[rows].reshape(1, DLOC), (128, DLOC))
                ),
                **shared,
            }
        )

    trace = bool(int(os.environ.get("KERNEL_TRACE", "0")))
    res = run_bass_kernel_spmd(nc, in_maps, list(range(NCORES)), trace=trace)
    if trace and res.exec_time_ns is not None:
        print(f"HW exec time: {res.exec_time_ns} ns")
        globals()["LAST_EXEC_NS"] = res.exec_time_ns
        globals()["LAST_RESULTS"] = res

    out = np.zeros((B, S, H), np.float32)
    for c in range(NCORES):
        out[c // HG] += res.results[c]["y"].reshape(S, H).astype(np.float32)
    out += bo
    return out


# revision 46
# speedup vs baseline: 1.2162x; 1.2162x over previous
"""Trainium2 Bass kernel: causal multi-head attention block (B=2,S=2048,H=2048,NH=16,HD=128).

Sharding: 8 cores = DP over batch (2) x TP over head-groups (4 groups of 4 heads).
Each core computes q/k/v projections for its 4 heads, RoPE, causal softmax
attention, and a partial output projection; the host sums the 4 partials per
batch and adds bo.

Single fused pipeline (587us 3-phase baseline -> ~347us). For causal
attention, q-tile i only attends k-tiles <= i, so one pass over the 4 s-tiles
of 512 suffices: iteration i runs attention for q-tile i while the projection
chains for s-tile i+1 and the (deferred) output projection for q-tile i-1 are
interleaved as PE filler work at sub-head granularity. This keeps the tensor
engine dense (the 3-phase baseline's PE gaps put the HAM clock-gate at 1.2GHz
for 208us; here cold time is ~12us), loads x once instead of twice, and makes
every DMA a contiguous pre-arranged block. Tile's HWDGE completion-sem lanes
are restored to 8 (a 1-lane config serializes every DMA on the ~2.2us
completion round-trip). Softmax denominators come from an f16 running sum of
the exp tiles on the DVE plus a single ones-matmul per (head, q-tile) instead
of a ones-matmul per k-tile (-62k PE cycles). Per head, the half-masked
diagonal k-tiles run FIRST so their ST->dbias(DVE)->exp(ACT) latency drains
under the dense unmasked stretch, and each unit's RoPE tail is emitted one
unit late so its rmat matmul never trails the ACT drain it depends on. y is
stored f16 (host upcasts and reduces the 4 TP partials).

Device layouts (host pre-arranges; matmul operands bf16, accumulation f32):
  x_pre  [4, 128, 16, 512]  x[b].T tiled [sj][fpart][ftile][s]      bf16
  wq/wk  [4, 128, 16, 128]  W.T tiled by head [h][fpart][ftile][d]  bf16
  wv     [128, 16, 512]     Wv.T tiled [fpart][ftile][d]            bf16
  wo     [128, 4, 2048]     Wo[:,rows].T tiled [dpart][dtile][o]    bf16
  cos/sin [4, 128, 512]     RoPE tables per s-tile [d][s]           f32
  y      [4, 4, 128, 2048]  [qj][ss][qpart][o]                      f16
Attention uses transposed score tiles ST[k,q] so P@V needs no transposes;
1/denom is broadcast across partitions with a K=1 matmul.
"""

import math
import os
import sys

import numpy as np

for _p in ("/opt/trn_rl_repo",):
    if _p not in sys.path and os.path.isdir(_p):
        sys.path.insert(0, _p)

import ml_dtypes

import concourse.bass as bass
import concourse.mybir as mybir
import concourse.tile as tile
from concourse import bacc

B, S, H, NH, HD = 2, 2048, 2048, 16, 128
NCORES = 8
HG = 4            # head-groups (TP degree)
HPG = NH // HG    # heads per group = 4
DLOC = HPG * HD   # local d width = 512
FT = H // 128     # 16 f-tiles
SJ = S // 512     # 4 s/q tiles of 512
KT128 = S // 128  # 16 k-tiles of 128
NEG = -1e30

F32 = mybir.dt.float32
F16 = mybir.dt.float16
BF16 = mybir.dt.bfloat16
NPBF16 = ml_dtypes.bfloat16
AF = mybir.ActivationFunctionType


def build_program(mode: str) -> bass.Bass:
    """mode in {'causal', 'full', 'bias'}"""
    # Tile enforces FIFO retirement per DMA-completion sem lane by making each
    # DMA issue wait for the previous one on its lane: with 1 lane every DMA
    # serializes on the ~2.2us completion round-trip (measured: the whole
    # bootstrap burst trickled at one transfer per 2.2us). 8 lanes allow 8
    # DMAs in flight. (The old "Too many sync wait commands" overflow that
    # forced 1 lane in the phase-structured kernel does not trigger with this
    # program shape.)
    import concourse.tile_sem_assignment as tsa

    tsa.NUM_HWDGE_SEMS = 8
    tsa.NUM_SWDGE_GLOBAL_SEMS = 1
    nc = bacc.Bacc()
    x_pre = nc.dram_tensor("x_pre", [SJ, 128, FT, 512], BF16, kind="ExternalInput")
    wq = nc.dram_tensor("wq", [HPG, 128, FT, 128], BF16, kind="ExternalInput")
    wk = nc.dram_tensor("wk", [HPG, 128, FT, 128], BF16, kind="ExternalInput")
    wv = nc.dram_tensor("wv", [128, FT, DLOC], BF16, kind="ExternalInput")
    wo = nc.dram_tensor("wo", [128, HPG, H], BF16, kind="ExternalInput")
    bqT = nc.dram_tensor("bqT", [128, HPG], F32, kind="ExternalInput")
    bkT = nc.dram_tensor("bkT", [128, HPG], F32, kind="ExternalInput")
    bvb = nc.dram_tensor("bvb", [128, DLOC], BF16, kind="ExternalInput")
    cosp = nc.dram_tensor("cosp", [SJ, HD, 512], F32, kind="ExternalInput")
    sinp = nc.dram_tensor("sinp", [SJ, HD, 512], F32, kind="ExternalInput")
    rmat = nc.dram_tensor("rmat", [HD, HD], BF16, kind="ExternalInput")
    onesd = nc.dram_tensor("onesd", [128, 1], F16, kind="ExternalInput")
    onesrd = nc.dram_tensor("onesrd", [1, 128], F16, kind="ExternalInput")
    if mode == "causal":
        dbias = nc.dram_tensor("dbias", [128, 4, 512], BF16, kind="ExternalInput")
    elif mode == "bias":
        fbias = nc.dram_tensor("fbias", [S, S], F32, kind="ExternalInput")
    y = nc.dram_tensor("y", [SJ, 4, 128, H], F16, kind="ExternalOutput")

    causal = mode == "causal"

    from contextlib import ExitStack

    with tile.TileContext(nc) as tc:
        with ExitStack() as ctx:
            pool = lambda *a, **k: ctx.enter_context(tc.tile_pool(*a, **k))  # noqa: E731
            consts = pool(name="consts", bufs=1)
            wqp = pool(name="wqp", bufs=1)
            wkp = pool(name="wkp", bufs=1)
            wvp = pool(name="wvp", bufs=1)
            wop = pool(name="wop", bufs=1)
            xin = pool(name="xin", bufs=2)
            cs_pool = pool(name="cs", bufs=2)
            qt_pool = pool(name="qt", bufs=9 if causal else KT128)
            kt_pool = pool(name="kt", bufs=KT128)
            vt_pool = pool(name="vt", bufs=KT128)
            rtmp_pool = pool(name="rtmp", bufs=2)
            e_pool = pool(name="ep", bufs=5)
            ee_pool = pool(name="eep", bufs=2)
            rc_pool = pool(name="rc", bufs=2)
            ot_pool = pool(name="ot", bufs=9)
            y_pool = pool(name="ysb", bufs=3)
            fb_pool = pool(name="fb", bufs=3)
            acc_psum = pool(name="acc", bufs=2, space="PSUM")
            st_psum = pool(name="stp", bufs=2, space="PSUM")
            pv_psum = pool(name="pvp", bufs=2, space="PSUM")
            yp_psum = pool(name="ypp", bufs=2, space="PSUM")
            rmat_sb = consts.tile([HD, HD], BF16, tag="rmat")
            bq_sb = consts.tile([128, HPG], F32, tag="bq")
            bk_sb = consts.tile([128, HPG], F32, tag="bk")
            ones_sb = consts.tile([128, 1], F16, tag="ones")
            onesr_sb = consts.tile([1, 128], F16, tag="onesr")

            # ---- bootstrap DMAs, ordered to match PE consumption order:
            # q-chains by head (wq per-head chunks interleaved with x0
            # quarters), then RoPE tables, k h0, the full wv (attention(0)
            # needs all V tiles first), remaining wk heads, then the
            # iteration-0 prefetches. ----
            wq_sb = wqp.tile([128, HPG, FT, 128], BF16, tag="wq")
            wk_sb = wkp.tile([128, HPG, FT, 128], BF16, tag="wk")
            wv_sb = wvp.tile([128, FT, DLOC], BF16, tag="wv")
            wo_sb = wop.tile([128, HPG, H], BF16, tag="wo")
            bv_sb = consts.tile([128, DLOC], BF16, tag="bv")
            db_sb = None
            if causal:
                db_sb = consts.tile([128, 4, 512], BF16, tag="db")

            # ---- PE warmup: the first ~14us are DMA-paced, so the HAM
            # clock-gate would hold the PE at 1.2GHz when real chains start.
            # A dependency-free matmul burst keeps the PE busy through the
            # activity window so the projections run at 2.4GHz. The chain is
            # drained by one DVE copy so its PSUM slot returns to the pool.
            warm_src = rtmp_pool.tile([128, 512], BF16, tag="warm", name="warm")
            nc.vector.memset(warm_src[:], 0.0)
            warm_ps = yp_psum.tile([128, 512], F32, tag="yp", name="warm_ps")
            NWARM = 14
            for wi in range(NWARM):
                nc.tensor.matmul(
                    warm_ps[:],
                    warm_src[:, 0:128],
                    warm_src[:],
                    start=(wi == 0),
                    stop=(wi == NWARM - 1),
                )
            warm_out = rtmp_pool.tile([128, 512], BF16, tag="warm", name="warm")
            nc.vector.tensor_copy(warm_out[:], warm_ps[:])

            XT = {}
            COS = {}
            SIN = {}
            XT[0] = xin.tile([128, FT, 512], BF16, tag="xt", name="xt")
            # rmat/biases are 34KiB and gate the first deferred-RoPE rmat
            # matmul (popped during q1's chain) -- they lead the FIFO
            nc.sync.dma_start(rmat_sb[:], rmat[:])
            nc.sync.dma_start(bq_sb[:], bqT[:])
            nc.sync.dma_start(bk_sb[:], bkT[:])
            nc.sync.dma_start(wq_sb[:, 0], wq[0])
            nc.sync.dma_start(XT[0][:, 0:2, :], x_pre[0, :, 0:2, :])
            nc.sync.dma_start(XT[0][:, 2:4, :], x_pre[0, :, 2:4, :])
            nc.sync.dma_start(XT[0][:, 4:6, :], x_pre[0, :, 4:6, :])
            nc.sync.dma_start(XT[0][:, 6:8, :], x_pre[0, :, 6:8, :])
            nc.sync.dma_start(ones_sb[:], onesd[:])
            nc.sync.dma_start(onesr_sb[:], onesrd[:])
            nc.sync.dma_start(wq_sb[:, 1], wq[1])
            nc.sync.dma_start(XT[0][:, 8:12, :], x_pre[0, :, 8:12, :])
            nc.sync.dma_start(wq_sb[:, 2], wq[2])
            nc.sync.dma_start(XT[0][:, 12:16, :], x_pre[0, :, 12:16, :])
            nc.sync.dma_start(wq_sb[:, 3], wq[3])
            COS[0] = cs_pool.tile([HD, 512], F32, tag="cos", name="cos")
            SIN[0] = cs_pool.tile([HD, 512], F32, tag="sin", name="sin")
            nc.sync.dma_start(COS[0][:], cosp[0])
            nc.sync.dma_start(SIN[0][:], sinp[0])
            nc.sync.dma_start(wk_sb[:, 0], wk[0])
            nc.sync.dma_start(bv_sb[:], bvb[:])
            nc.sync.dma_start(wv_sb[:], wv[:])
            for hh in range(1, HPG):
                nc.sync.dma_start(wk_sb[:, hh], wk[hh])
            if causal:
                nc.sync.dma_start(db_sb[:], dbias[:])
            # prefetch s-tile 1 inputs during the bootstrap projections
            XT[1] = xin.tile([128, FT, 512], BF16, tag="xt", name="xt")
            nc.sync.dma_start(XT[1][:], x_pre[1])
            COS[1] = cs_pool.tile([HD, 512], F32, tag="cos", name="cos")
            SIN[1] = cs_pool.tile([HD, 512], F32, tag="sin", name="sin")
            nc.sync.dma_start(COS[1][:], cosp[1])
            nc.sync.dma_start(SIN[1][:], sinp[1])
            nc.sync.dma_start(wo_sb[:], wo[:])

            QT = {}   # (h, sj) -> [128(d), 512(s)] bf16, RoPE'd q^T (pre-scaled)
            KT = {}   # (h, sj) -> [128(d), 512(s)] bf16, RoPE'd k^T
            VT = {}   # ss -> [128(s), 512(d)] bf16, v + bias
            PV = {}   # h -> psum [128(d), 512(q)] accumulated P@V
            EE = {}   # h -> [128(k), 512(q)] f16 running sum of exp tiles
            RCH = {}  # h -> [1, 512] f16 reciprocal denominators
            OT = {}   # h -> [128(d), 512(q)] bf16 normalized attention out

            PENDING = []  # deferred RoPE emissions (closures)

            def proj_qk(which, h, i, defer_rope=True):
                """Projection chain for (q|k, head h, s-tile i); the RoPE tail
                (rmat matmul + DVE muls) is deferred so the rmat matmul is not
                emitted right behind the ACT drain it depends on -- the next
                unit's chain runs in between instead of the PE stalling."""
                w_sb, b_sb = (wq_sb, bq_sb) if which == "q" else (wk_sb, bk_sb)
                store = QT if which == "q" else KT
                pool = qt_pool if which == "q" else kt_pool
                ps = acc_psum.tile([128, 512], F32, tag="acc", name="acc")
                for ft in range(FT):
                    nc.tensor.matmul(
                        ps[:],
                        w_sb[:, h, ft, :],
                        XT[i][:, ft, :],
                        start=(ft == 0),
                        stop=(ft == FT - 1),
                    )
                t = pool.tile([128, 512], BF16, tag="t", name="qkt")
                nc.scalar.activation(t[:], ps[:], AF.Identity, bias=b_sb[:, h : h + 1])
                store[(h, i)] = t

                def rope():
                    rp = st_psum.tile([128, 512], F32, tag="st", name="rp")
                    nc.tensor.matmul(rp[:], rmat_sb[:], t[:], start=True, stop=True)
                    tmp = rtmp_pool.tile([128, 512], BF16, tag="tmp", name="tmp")
                    nc.vector.tensor_mul(tmp[:], rp[:], SIN[i][:])
                    nc.vector.tensor_mul(t[:], t[:], COS[i][:])
                    nc.vector.tensor_add(t[:], t[:], tmp[:])

                if defer_rope:
                    PENDING.append(rope)
                else:
                    rope()

            def proj_v(u, i):
                """V projection for s-subtile 128*(4i+u)."""
                ss = 4 * i + u
                ps = acc_psum.tile([128, 512], F32, tag="acc", name="acc")
                for ft in range(FT):
                    nc.tensor.matmul(
                        ps[:],
                        XT[i][:, ft, u * 128 : (u + 1) * 128],
                        wv_sb[:, ft, :],
                        start=(ft == 0),
                        stop=(ft == FT - 1),
                    )
                v = vt_pool.tile([128, DLOC], BF16, tag="v", name="v")
                nc.vector.tensor_add(v[:], ps[:], bv_sb[:])
                VT[ss] = v

            def emit_unit(unit, i):
                which, idx = unit
                if PENDING:
                    PENDING.pop(0)()
                if which == "v":
                    proj_v(idx, i)
                else:
                    proj_qk(which, idx, i)

            def denom(h):
                """Start the denominator chain for head h (dn matmul+recip).

                dn lives in the acc pool: during attention the projection
                accumulators are mostly idle, while the st pool's two banks
                are kept saturated by the ST pipeline."""
                dn = acc_psum.tile([1, 512], F32, tag="acc", name="dn")
                nc.tensor.matmul(dn[:], ones_sb[:], EE[h][:], start=True, stop=True)
                rcf = rc_pool.tile([1, 512], F32, tag="rcf", name="rcf")
                nc.vector.reciprocal_approx_fast(rcf[:], dn[:])
                rch = rc_pool.tile([1, 512], F16, tag="rch", name="rch")
                nc.vector.tensor_copy(rch[:], rcf[:])
                RCH[h] = rch

            def normalize(i, h):
                """Broadcast 1/denom and scale P@V for head h."""
                rcb_ps = yp_psum.tile([128, 512], F32, tag="yp", name="rcb_ps")
                nc.tensor.matmul(rcb_ps[:], onesr_sb[:], RCH[h][:], start=True, stop=True)
                rcb = rc_pool.tile([128, 512], F16, tag="rcb", name="rcb")
                nc.vector.tensor_copy(rcb[:], rcb_ps[:])
                ot = ot_pool.tile([128, 512], BF16, tag="ot", name="ot")
                nc.vector.tensor_mul(ot[:], PV[h][:], rcb[:])
                OT[(i, h)] = ot

            PRE_E = {}  # h -> pre-emitted exp tile for that head's pos 0

            def attn_pre(i, h):
                """Pre-emit head h's first ST+exp (always the full-width a=0
                diagonal tile) so ACT computes it during the preceding filler
                block instead of stalling the first PV matmul."""
                kj0 = 4 * i if causal else 0
                st = st_psum.tile([128, 512], F32, tag="st", name="st")
                nc.tensor.matmul(
                    st[:],
                    KT[(h, kj0 // 4)][:, (kj0 % 4) * 128 : (kj0 % 4 + 1) * 128],
                    QT[(h, i)][:],
                    start=True,
                    stop=True,
                )
                if causal:
                    nc.vector.tensor_add(
                        st[:, 0:128], st[:, 0:128], db_sb[:, 0, 0:128]
                    )
                elif mode == "bias":
                    fb = fb_pool.tile([128, 512], F32, tag="fb", name="fb")
                    nc.sync.dma_start(
                        fb[:], fbias[kj0 * 128 : (kj0 + 1) * 128, i * 512 : (i + 1) * 512]
                    )
                    nc.vector.tensor_add(st[:], st[:], fb[:])
                e = e_pool.tile([128, 512], BF16, tag="e", name="e")
                nc.scalar.activation(e[:], st[:], AF.Exp)
                PRE_E[h] = e

            def attn_head(i, h, fillers=None):
                """Causal attention for (q-tile i, head h); ST[k,q] layout.

                The denom/normalize work for head h-1 is sprinkled after
                kj==0 and kj==1 so the PE never waits on the DVE reciprocal
                chain.  `fillers` is a list of callables emitting independent
                PE work (deferred oproj chains); one is drained every 4
                k-tiles to cover the ~200ns/k-tile exp-gated PV stall in the
                last iteration.
                """
                kmax = 4 * i + 4 if causal else KT128
                # Diagonal k-tiles go FIRST: their ST -> dbias(DVE) ->
                # exp(ACT) -> PV chain has two engine hops, so their latency
                # drains under the dense unmasked stretch instead of piling
                # up at the head's end right before the denominator chain.
                # PSUM accumulation order is commutative; position 0 is always
                # full-width (a=0), so the ee running sum starts with a copy.
                if causal:
                    kjs = list(range(4 * i, kmax)) + list(range(0, 4 * i))
                else:
                    kjs = list(range(kmax))
                pv = pv_psum.tile([128, 512], F32, tag="pv", name="pv")
                ee = ee_pool.tile([128, 512], F16, tag="ee", name="ee")
                for pos, kj in enumerate(kjs):
                    a = kj - 4 * i
                    off = 128 * a if (causal and a > 0) else 0
                    if pos == 0 and h in PRE_E:
                        # ST+exp for pos 0 were pre-emitted before the
                        # preceding filler block; ACT already computed them.
                        e = PRE_E.pop(h)
                    else:
                        st = st_psum.tile([128, 512], F32, tag="st", name="st")
                        nc.tensor.matmul(
                            st[:, off:],
                            KT[(h, kj // 4)][:, (kj % 4) * 128 : (kj % 4 + 1) * 128],
                            QT[(h, i)][:, off:],
                            start=True,
                            stop=True,
                        )
                        if causal and a >= 0:
                            nc.vector.tensor_add(
                                st[:, off : off + 128],
                                st[:, off : off + 128],
                                db_sb[:, a, off : off + 128],
                            )
                        elif mode == "bias":
                            fb = fb_pool.tile([128, 512], F32, tag="fb", name="fb")
                            nc.sync.dma_start(
                                fb[:],
                                fbias[
                                    kj * 128 : (kj + 1) * 128,
                                    i * 512 : (i + 1) * 512,
                                ],
                            )
                            nc.vector.tensor_add(st[:], st[:], fb[:])
                        e = e_pool.tile([128, 512], BF16, tag="e", name="e")
                        nc.scalar.activation(e[:, off:], st[:, off:], AF.Exp)
                    # independent PE work emitted BETWEEN the exp and the PV
                    # matmul that waits on it, covering the ACT latency
                    if h > 0 and pos == 0:
                        denom(h - 1)
                    if h > 0 and pos == 1:
                        normalize(i, h - 1)
                    if fillers and len(fillers) > 2 and pos % 4 == 3:
                        fillers.pop(0)()
                    nc.tensor.matmul(
                        pv[:, off:],
                        VT[kj][:, h * 128 : (h + 1) * 128],
                        e[:, off:],
                        start=(pos == 0),
                        stop=(pos == kmax - 1),
                    )
                    if pos == 0:
                        nc.vector.tensor_copy(ee[:], e[:])
                    else:
                        nc.vector.tensor_add(ee[:, off:], ee[:, off:], e[:, off:])
                PV[h] = pv
                EE[h] = ee

            def oproj(i, ss, split_store=False):
                """Output projection rows 512i+128ss..+128, all 2048 cols."""
                ysb = y_pool.tile([128, H], F16, tag="y", name="y")
                for oj in range(4):
                    yp = yp_psum.tile([128, 512], F32, tag="yp", name="yp")
                    for dt in range(HPG):
                        nc.tensor.matmul(
                            yp[:],
                            OT[(i, dt)][:, ss * 128 : (ss + 1) * 128],
                            wo_sb[:, dt, oj * 512 : (oj + 1) * 512],
                            start=(dt == 0),
                            stop=(dt == HPG - 1),
                        )
                    dst = ysb[:, oj * 512 : (oj + 1) * 512]
                    if oj % 2 == 0:
                        nc.scalar.activation(dst, yp[:], AF.Identity)
                    else:
                        nc.vector.tensor_copy(dst, yp[:])
                    if split_store:
                        nc.sync.dma_start(y[i, ss, :, oj * 512 : (oj + 1) * 512], dst)
                if not split_store:
                    nc.sync.dma_start(y[i, ss], ysb[:])

            def oproj_pieces(i):
                """oproj(i) as 16 single (ss, oj) chain emissions for use as
                in-loop PE filler."""
                holder = {}

                def make(ss, oj):
                    def f():
                        if oj == 0:
                            holder[ss] = y_pool.tile([128, H], F16, tag="y", name="y")
                        ysb = holder[ss]
                        yp = yp_psum.tile([128, 512], F32, tag="yp", name="yp")
                        for dt in range(HPG):
                            nc.tensor.matmul(
                                yp[:],
                                OT[(i, dt)][:, ss * 128 : (ss + 1) * 128],
                                wo_sb[:, dt, oj * 512 : (oj + 1) * 512],
                                start=(dt == 0),
                                stop=(dt == HPG - 1),
                            )
                        dst = ysb[:, oj * 512 : (oj + 1) * 512]
                        if oj % 2 == 0:
                            nc.scalar.activation(dst, yp[:], AF.Identity)
                        else:
                            nc.vector.tensor_copy(dst, yp[:])
                        if oj == 3:
                            nc.sync.dma_start(y[i, ss], ysb[:])

                    return f

                return [make(ss, oj) for ss in range(4) for oj in range(4)]

            def flush_pending():
                while PENDING:
                    PENDING.pop(0)()

            # ---- projections for s-tile 0, in DMA-arrival order ----
            for u0 in [("q", 0), ("q", 1), ("q", 2), ("q", 3), ("k", 0),
                       ("v", 0), ("v", 1), ("v", 2), ("v", 3),
                       ("k", 1), ("k", 2), ("k", 3)]:
                emit_unit(u0, 0)
            flush_pending()

            # For full/bias modes every q-tile needs all k-tiles, so emit all
            # projections up front and skip the interleaving.
            if not causal:
                for i in range(1, SJ):
                    XT[i] = XT.get(i) or xin.tile(
                        [128, FT, 512], BF16, tag="xt", name="xt"
                    )
                    if i > 1:
                        nc.sync.dma_start(XT[i][:], x_pre[i])
                        COS[i] = cs_pool.tile([HD, 512], F32, tag="cos", name="cos")
                        SIN[i] = cs_pool.tile([HD, 512], F32, tag="sin", name="sin")
                        nc.sync.dma_start(COS[i][:], cosp[i])
                        nc.sync.dma_start(SIN[i][:], sinp[i])
                    for h in range(HPG):
                        proj_qk("q", h, i, defer_rope=False)
                    for h in range(HPG):
                        proj_qk("k", h, i, defer_rope=False)
                    for u in range(4):
                        proj_v(u, i)

            UNITS = [("q", 0), ("q", 1), ("q", 2), ("q", 3),
                     ("k", 0), ("k", 1), ("k", 2), ("k", 3),
                     ("v", 0), ("v", 1), ("v", 2), ("v", 3)]

            for i in range(SJ):
                nxt = i + 1
                fill = causal and nxt < SJ
                if fill:
                    # prefetch s-tile i+2 inputs; the i+1 tiles landed during
                    # the previous iteration.
                    if nxt + 1 < SJ:
                        j = nxt + 1
                        COS[j] = cs_pool.tile([HD, 512], F32, tag="cos", name="cos")
                        SIN[j] = cs_pool.tile([HD, 512], F32, tag="sin", name="sin")
                        nc.sync.dma_start(COS[j][:], cosp[j])
                        nc.sync.dma_start(SIN[j][:], sinp[j])
                        XT[j] = xin.tile([128, FT, 512], BF16, tag="xt", name="xt")
                        nc.sync.dma_start(XT[j][:], x_pre[j])

                # oproj for q-tile i-1 is deferred into this iteration: its
                # matmul chains have no ACT/DVE dependencies, so they fill the
                # PE while the exp chain gates this tile's PV matmuls.  In the
                # last iteration (no projection units left) the pieces go
                # INSIDE the kj loops, where the exp-gated stalls actually are.
                last = causal and i == SJ - 1
                fillers = oproj_pieces(i - 1) if last else None
                attn_head(i, 0, fillers)
                attn_pre(i, 1)
                if causal and 0 < i < SJ - 1:
                    oproj(i - 1, 0)
                if fill:
                    emit_unit(UNITS[0], nxt)
                attn_head(i, 1, fillers)
                attn_pre(i, 2)
                if causal and 0 < i < SJ - 1:
                    oproj(i - 1, 1)
                if fill:
                    emit_unit(UNITS[1], nxt)
                attn_head(i, 2, fillers)
                attn_pre(i, 3)
                if causal and 0 < i < SJ - 1:
                    oproj(i - 1, 2)
                if fill:
                    emit_unit(UNITS[2], nxt)
                    emit_unit(UNITS[3], nxt)
                attn_head(i, 3, fillers)
                if causal and 0 < i < SJ - 1:
                    oproj(i - 1, 3)
                if fill:
                    emit_unit(UNITS[4], nxt)
                denom(3)
                if fill:
                    emit_unit(UNITS[5], nxt)
                elif fillers:
                    # last iteration: a reserved oproj piece covers the PE
                    # while the DVE reciprocal chain for head 3 completes
                    fillers.pop(0)()
                normalize(i, 3)
                if fillers:
                    while fillers:
                        fillers.pop(0)()
                if not causal:
                    for ss in range(4):
                        oproj(i, ss)
                if fill:
                    emit_unit(UNITS[6], nxt)
                    emit_unit(UNITS[7], nxt)
                    emit_unit(UNITS[8], nxt)
                    emit_unit(UNITS[9], nxt)
                    emit_unit(UNITS[10], nxt)
                    emit_unit(UNITS[11], nxt)
                    flush_pending()
            if causal:
                for ss in range(4):
                    oproj(SJ - 1, ss, split_store=True)
    nc.compile()
    return nc


_PROGRAM_CACHE = {}


def _get_program(mode):
    if mode not in _PROGRAM_CACHE:
        _PROGRAM_CACHE[mode] = build_program(mode)
    return _PROGRAM_CACHE[mode]


def _detect_mode(attn_mask):
    m = np.asarray(attn_mask).reshape(S, S)
    if (m == np.tril(np.ones((S, S), m.dtype))).all():
        return "causal"
    if (m != 0).all():
        return "full"
    return "bias"


def _rot_matrix():
    # rot(q)[d'] = -q[d'+1] (d' even), +q[d'-1] (d' odd);  rotT = R^T @ qT with
    # lhsT[d, d'] convention of nc.tensor.matmul.
    r = np.zeros((HD, HD), np.float32)
    for dp in range(HD):
        if dp % 2 == 0:
            r[dp + 1, dp] = -1.0
        else:
            r[dp - 1, dp] = 1.0
    return r


def _diag_bias():
    # [p, a, t]: 0 where k=128a+p visible to q=t within the diagonal band
    a = np.arange(4)[None, :, None]
    p = np.arange(128)[:, None, None]
    t = np.arange(512)[None, None, :]
    return np.where(128 * a + p <= t, 0.0, NEG).astype(NPBF16)


def _bf16(a):
    return np.ascontiguousarray(a).astype(NPBF16)


def kernel(**inputs) -> np.ndarray:
    from concourse.bass_utils import run_bass_kernel_spmd

    x = np.asarray(inputs["x"], np.float32)
    fcos = np.asarray(inputs["fcos"], np.float32)
    fsin = np.asarray(inputs["fsin"], np.float32)
    Wq, bq = np.asarray(inputs["Wq"], np.float32), np.asarray(inputs["bq"], np.float32)
    Wk, bk = np.asarray(inputs["Wk"], np.float32), np.asarray(inputs["bk"], np.float32)
    Wv, bv = np.asarray(inputs["Wv"], np.float32), np.asarray(inputs["bv"], np.float32)
    Wo, bo = np.asarray(inputs["Wo"], np.float32), np.asarray(inputs["bo"], np.float32)
    attn_mask = inputs["attn_mask"]

    mode = _detect_mode(attn_mask)
    nc = _get_program(mode)

    sc = 1.0 / math.sqrt(HD)
    shared = {
        "cosp": np.ascontiguousarray(
            fcos.T.reshape(HD, SJ, 512).transpose(1, 0, 2)
        ).astype(np.float32),
        "sinp": np.ascontiguousarray(
            fsin.T.reshape(HD, SJ, 512).transpose(1, 0, 2)
        ).astype(np.float32),
        "rmat": _rot_matrix().astype(NPBF16),
        "onesd": np.ones((128, 1), np.float16),
        "onesrd": np.ones((1, 128), np.float16),
    }
    if mode == "causal":
        shared["dbias"] = _diag_bias()
    elif mode == "bias":
        m = np.asarray(attn_mask).reshape(S, S)
        shared["fbias"] = np.ascontiguousarray(
            np.where(m.T == 0, NEG, 0.0).astype(np.float32)
        )

    in_maps = []
    for c in range(NCORES):
        b, hg = divmod(c, HG)
        rows = slice(DLOC * hg, DLOC * (hg + 1))
        xT = x[b].T  # [H, S]
        in_maps.append(
            {
                "x_pre": _bf16(
                    xT.reshape(FT, 128, SJ, 512).transpose(2, 1, 0, 3)
                ),
                "wq": _bf16(
                    (Wq[rows] * sc).T.reshape(FT, 128, HPG, 128).transpose(2, 1, 0, 3)
                ),
                "wk": _bf16(
                    Wk[rows].T.reshape(FT, 128, HPG, 128).transpose(2, 1, 0, 3)
                ),
                "wv": _bf16(Wv[rows].T.reshape(FT, 128, DLOC).transpose(1, 0, 2)),
                "wo": _bf16(Wo[:, rows].T.reshape(HPG, 128, H).transpose(1, 0, 2)),
                "bqT": np.ascontiguousarray((bq[rows] * sc).reshape(HPG, 128).T),
                "bkT": np.ascontiguousarray(bk[rows].reshape(HPG, 128).T),
                "bvb": _bf16(
                    np.broadcast_to(bv[rows].reshape(1, DLOC), (128, DLOC))
                ),
                **shared,
            }
        )

    trace = bool(int(os.environ.get("KERNEL_TRACE", "0")))
    res = run_bass_kernel_spmd(nc, in_maps, list(range(NCORES)), trace=trace)
    if trace and res.exec_time_ns is not None:
        print(f"HW exec time: {res.exec_time_ns} ns")
        globals()["LAST_EXEC_NS"] = res.exec_time_ns
        globals()["LAST_RESULTS"] = res

    out = np.zeros((B, S, H), np.float32)
    for c in range(NCORES):
        out[c // HG] += res.results[c]["y"].reshape(S, H).astype(np.float32)
    out += bo
    return out
